# revision 1
# baseline (speedup 1.0000x reference)
"""AnchorAttention distributed Bass kernel for 8 TRN2 NeuronCores.

Problem: x:(2, 8192, 1024) f32; first A=1024 tokens per batch are anchors.
  aqkv = anchors @ Wqkv -> per-head aq, ak, av     (H=16 heads, hd=64)
  qq   = queries @ Wq   -> per-head q
  out  = softmax((concat(aq,qq) @ ak^T)/8) @ av, reshaped, @ Wproj

Sharding: sequence-parallel. Core c in 0..7 owns 2048 token rows:
  batch b = c // 4, rows [2048*(c%4), 2048*(c%4)+2048) of that batch.
Each query row attends only to its batch's anchors, so no collectives are
needed; anchor K/V is recomputed on each core (small). Host pre-transposes
and pre-casts inputs to bf16; rel-err tolerance is ~2e-2 and bf16 compute
lands ~3e-3.

Device kernel layout choices (everything transposed so matmuls chain):
  xT (D, 2048), aT (D, 1024) in DRAM.
  KT[dk, a]   = (anchors @ Wk)^T        via out^T = WkT-tile-stationary mm
  V[a, dv]    = anchors @ Wv            natural
  QT[dq, q]   = (rows @ Wq_eff)^T
  scores^T[a_tile, q] = KT-tile^T @ QT  (K=hd=64; even/odd heads auto
                       row-tile to array rows 0-63 / 64-127 and write
                       different PSUM banks)
  ST = exp(scores * 1/8)  on ScalarE, PSUM->SBUF bf16   (max-free softmax:
                       scores are O(+-8) so exp is safe in f32/bf16)
  attn-out^T[hd, q] accumulated over anchor tiles: lhsT = V tile, the head
                       pair col-packed via tile_position (0,0)/(0,64)
  denom[q] = ones^T @ ST (M=1 matmuls, col-packed to psum partitions 0/64);
  both PSUM accumulators are evacuated immediately (copy / reciprocal) so
  the next pair never stalls on the normalize chain; recips are partition-
  broadcast via a DRAM bounce (stride-0 partition DMA is DRAM-source only)
  and applied as one tensor_tensor multiply into AO.
  final out[q, do]: lhsT = AO tile, rhs = Wproj, f32 DMA out.

  The whole kernel is software-pipelined in 4 q-rounds of 512 columns:
  round qb interleaves attention (ACT-bound: 64 exps of [128,1024]) with
  the NEXT round's Q-projection matmuls and the PREVIOUS round's output-
  projection matmuls, so the PE work of those phases hides under the
  ScalarE exp stream. KT projection rides inside round 0; the V projection
  and first Q-projection block form the prologue.
"""

import sys

if "/opt/trn_rl_repo" not in sys.path:
    sys.path.insert(0, "/opt/trn_rl_repo")

import numpy as np
import ml_dtypes

from concourse import bacc, mybir, tile
from concourse.bass_utils import run_bass_kernel_spmd

# ---------------------------------------------------------------- constants
B, S, D = 2, 8192, 1024
H, HD, A = 16, 64, 1024
NQ = 2048          # token rows per core
NCORES = 8
DT = D // 128      # 8 x 128-row tiles of the model dim
AT = A // 128      # 8 anchor tiles
QB = 512           # q block inside attention
NPAIR = H // 2     # head pairs (adjacent heads share a 128-partition tile)
SCALE = 1.0 / 8.0  # 1/sqrt(hd)

F32 = mybir.dt.float32
BF16 = mybir.dt.bfloat16

_cached_nc = None

BIGEXP = False  # exp over [128, 2048] spanning two anchor tiles (slower)
COLSCORES = False  # scores as M=64-split col-mode mms (no PE mode switches)


def build_kernel(repeat=1):
    nc = bacc.Bacc("TRN2", target_bir_lowering=False, debug=False,
                   num_devices=NCORES)

    xT = nc.declare_dram_parameter("xT", [D, NQ], BF16, isOutput=False)
    aT = nc.declare_dram_parameter("aT", [D, A], BF16, isOutput=False)
    wq0 = nc.declare_dram_parameter("wq0", [D, D], BF16, isOutput=False)
    wq1 = nc.declare_dram_parameter("wq1", [D, D], BF16, isOutput=False)
    wkv = nc.declare_dram_parameter("wkv", [D, 2 * D], BF16, isOutput=False)
    wpr = nc.declare_dram_parameter("wpr", [D, D], BF16, isOutput=False)
    # Q bias, column 0 for q<1024 rows, column 1 for the rest (exact, f32)
    bq2 = nc.declare_dram_parameter("bq2", [D, 2], F32, isOutput=False)
    out = nc.declare_dram_parameter("out", [NQ, D], F32, isOutput=True)

    NQB = NQ // QB  # 4 q rounds

    with tile.TileContext(nc) as tc:
        for _rep in range(repeat):
          with (
            tc.tile_pool(name="attn", bufs=1) as p_attn,      # KT, V, QT
            tc.tile_pool(name="ao", bufs=1) as p_ao,          # AO blocks, WP
            tc.tile_pool(name="stage", bufs=1) as p_stage,    # aT
            tc.tile_pool(name="xq", bufs=2) as p_xq,          # x panel / round
            tc.tile_pool(name="wt", bufs=2) as p_w,           # weight panels
            tc.tile_pool(name="st", bufs=(2 if BIGEXP else 4)) as p_st,
            tc.tile_pool(name="small", bufs=1) as p_small,
            tc.tile_pool(name="pvs", bufs=4) as p_pvs,
            tc.tile_pool(name="rcb", bufs=3) as p_rcb,
            tc.tile_pool(name="scr", bufs=6, space="DRAM") as p_scr,
            tc.tile_pool(name="outsb", bufs=2) as p_out,
            tc.tile_pool(name="psps", bufs=(1 if BIGEXP else 2),
                         space="PSUM") as ps_s,
            tc.tile_pool(name="psacc", bufs=2, space="PSUM") as ps_acc,
            tc.tile_pool(name="psv", bufs=1, space="PSUM") as ps_v,
            tc.tile_pool(name="psd", bufs=1, space="PSUM") as ps_d,
          ):
            KT = p_attn.tile([128, DT, A], BF16, tag="KT")
            V = p_attn.tile([128, AT, D], BF16, tag="V")
            QT = p_attn.tile([128, DT, NQ], BF16, tag="QT")
            AOq = []
            for i in range(NQB):
                ao_i = p_ao.tile([128, DT, QB], BF16, tag=f"AO{i}",
                                 name=f"AO{i}")
                AOq.append(ao_i)
            WP = p_ao.tile([128, DT, D], BF16, tag="WP")
            ones = p_small.tile([128, 1], BF16, tag="ones")
            nc.vector.memset(ones[:], 1.0)
            bqs = p_small.tile([128, DT, 2], F32, tag="bqs")
            nc.sync.dma_start(
                out=bqs[:], in_=bq2[:].rearrange("(k p) c -> p k c", p=128))
            nc.gpsimd.dma_start(
                out=WP[:], in_=wpr[:].rearrange("(k p) c -> p k c", p=128))
            aTs = p_stage.tile([128, DT, A], BF16, tag="aT")
            nc.scalar.dma_start(
                out=aTs[:], in_=aT[:].rearrange("(k p) a -> p k a", p=128))

            # ---------------- emission helpers ---------------------------
            def emit_V_panel(vh):
                wv = p_w.tile([128, DT, 512], BF16, tag="wv", name=f"wv{vh}")
                nc.gpsimd.dma_start(
                    out=wv[:],
                    in_=wkv[:, D + vh * 512:D + (vh + 1) * 512].rearrange(
                        "(k p) c -> p k c", p=128))
                return wv

            def emit_V_at(vh, wv, at):
                acc = ps_acc.tile([128, 512], F32, tag="acc")
                for dn in range(DT):
                    nc.tensor.matmul(
                        acc[:],
                        lhsT=aTs[:, dn, at * 128:(at + 1) * 128],
                        rhs=wv[:, dn, :],
                        start=(dn == 0), stop=(dn == DT - 1))
                nc.vector.tensor_copy(
                    V[:, at, vh * 512:(vh + 1) * 512], acc[:])

            def emit_KT(dk):
                wk = p_w.tile([128, DT, 128], BF16, tag="wk")
                nc.gpsimd.dma_start(
                    out=wk[:],
                    in_=wkv[:, dk * 128:(dk + 1) * 128].rearrange(
                        "(k p) c -> p k c", p=128))
                for ah in range(2):
                    acc = ps_acc.tile([128, 512], F32, tag="acc")
                    for dn in range(DT):
                        nc.tensor.matmul(
                            acc[:],
                            lhsT=wk[:, dn, :],
                            rhs=aTs[:, dn, ah * 512:(ah + 1) * 512],
                            start=(dn == 0), stop=(dn == DT - 1))
                    nc.vector.tensor_copy(
                        KT[:, dk, ah * 512:(ah + 1) * 512], acc[:])

            def emit_xq(qs):
                xq = p_xq.tile([128, DT, QB], BF16, tag="xq")
                nc.scalar.dma_start(
                    out=xq[:],
                    in_=xT[:, qs * QB:(qs + 1) * QB].rearrange(
                        "(k p) q -> p k q", p=128))
                return xq

            def emit_C_dq(qs, dq, xq):
                half = 0 if qs < 2 else 1
                wsrc = wq0 if half == 0 else wq1
                wq_t = p_w.tile([128, DT, 128], BF16, tag="wq")
                nc.gpsimd.dma_start(
                    out=wq_t[:],
                    in_=wsrc[:, dq * 128:(dq + 1) * 128].rearrange(
                        "(k p) c -> p k c", p=128))
                acc = ps_acc.tile([128, 512], F32, tag="acc")
                for dn in range(DT):
                    nc.tensor.matmul(
                        acc[:],
                        lhsT=wq_t[:, dn, :],
                        rhs=xq[:, dn, :],
                        start=(dn == 0), stop=(dn == DT - 1))
                nc.vector.tensor_scalar_add(
                    QT[:, dq, qs * QB:(qs + 1) * QB], acc[:],
                    bqs[:, dq, half:half + 1])

            def emit_scores_exp(qb, g, at):
                ps = ps_s.tile([128, 2 * QB], F32, tag="ps")
                for hl in range(2):
                    nc.tensor.matmul(
                        ps[:, hl * QB:(hl + 1) * QB],
                        lhsT=KT[hl * 64:(hl + 1) * 64, g,
                                at * 128:(at + 1) * 128],
                        rhs=QT[hl * 64:(hl + 1) * 64, g,
                               qb * QB:(qb + 1) * QB],
                        start=True, stop=True)
                st = p_st.tile([128, 2 * QB], BF16, tag="st")
                nc.scalar.activation(
                    st[:], ps[:], mybir.ActivationFunctionType.Exp,
                    scale=SCALE)
                return st

            def emit_av_denom(g, at, st, pv, pd):
                for hl in range(2):
                    h = 2 * g + hl
                    nc.tensor.matmul(
                        pv[hl * 64:(hl + 1) * 64, :],
                        lhsT=V[:, at, h * 64:(h + 1) * 64],
                        rhs=st[:, hl * QB:(hl + 1) * QB],
                        start=(at == 0), stop=(at == AT - 1),
                        tile_position=(0, hl * 64))
                for hl in range(2):
                    nc.tensor.matmul(
                        pd[hl * 64:hl * 64 + 1, :],
                        lhsT=ones[:],
                        rhs=st[:, hl * QB:(hl + 1) * QB],
                        start=(at == 0), stop=(at == AT - 1),
                        tile_position=(0, hl * 64))

            def emit_normalize(qb, g, pv, pd):
                pvs = p_pvs.tile([128, QB], F32, tag="pvs")
                nc.vector.tensor_copy(pvs[:], pv[:])
                rc = p_pvs.tile([65, QB], F32, tag="rcp")
                nc.vector.reciprocal(rc[:], pd[0:65, :])
                scr = p_scr.tile([2, QB], F32, tag="scr")
                nc.sync.dma_start(out=scr[0:1, :], in_=rc[0:1, :])
                nc.sync.dma_start(out=scr[1:2, :], in_=rc[64:65, :])
                rb = p_rcb.tile([128, QB], F32, tag="rb")
                for hl in range(2):
                    nc.sync.dma_start(
                        out=rb[hl * 64:(hl + 1) * 64, :],
                        in_=scr[hl:hl + 1, :].to_broadcast((64, QB)))
                nc.vector.tensor_tensor(
                    out=AOq[qb][:, g, :], in0=pvs[:], in1=rb[:],
                    op=mybir.AluOpType.mult)

            def emit_attention(qb, g, at_hook=None):
                # pv: both heads stacked in partitions; pd: denoms at 0 / 64
                pv = ps_v.tile([128, QB], F32, tag="pv")
                pd = ps_d.tile([128, QB], F32, tag="pd")
                ATG = 2 if BIGEXP else 1   # anchor tiles per exp group
                for atg in range(AT // ATG):
                    if at_hook is not None:
                        for _a in range(atg * ATG, (atg + 1) * ATG):
                            at_hook(_a)
                    ps = ps_s.tile([128, ATG * 2 * QB], F32, tag="ps")
                    # scores^T: even head on PE rows 0-63, odd on 64-127
                    # (auto row-tiling), different PSUM banks.  COLSCORES
                    # instead splits M into two col-mode tiles so attention
                    # never changes PE tiling mode.
                    for ai in range(ATG):
                        at = atg * ATG + ai
                        for hl in range(2):
                            if COLSCORES:
                                for mh in range(2):
                                    nc.tensor.matmul(
                                        ps[mh * 64:(mh + 1) * 64,
                                           (2 * ai + hl) * QB:
                                           (2 * ai + hl + 1) * QB],
                                        lhsT=KT[hl * 64:(hl + 1) * 64, g,
                                                at * 128 + mh * 64:
                                                at * 128 + (mh + 1) * 64],
                                        rhs=QT[hl * 64:(hl + 1) * 64, g,
                                               qb * QB:(qb + 1) * QB],
                                        start=True, stop=True,
                                        tile_position=(0, mh * 64))
                            else:
                                nc.tensor.matmul(
                                    ps[:, (2 * ai + hl) * QB:
                                       (2 * ai + hl + 1) * QB],
                                    lhsT=KT[hl * 64:(hl + 1) * 64, g,
                                            at * 128:(at + 1) * 128],
                                    rhs=QT[hl * 64:(hl + 1) * 64, g,
                                           qb * QB:(qb + 1) * QB],
                                    start=True, stop=True)
                    st = p_st.tile([128, ATG * 2 * QB], BF16, tag="st")
                    nc.scalar.activation(
                        st[:], ps[:], mybir.ActivationFunctionType.Exp,
                        scale=SCALE)
                    # attn @ V (col-packed pair) + denominators (M=1)
                    for ai in range(ATG):
                        at = atg * ATG + ai
                        for hl in range(2):
                            h = 2 * g + hl
                            nc.tensor.matmul(
                                pv[hl * 64:(hl + 1) * 64, :],
                                lhsT=V[:, at, h * 64:(h + 1) * 64],
                                rhs=st[:, (2 * ai + hl) * QB:
                                       (2 * ai + hl + 1) * QB],
                                start=(at == 0), stop=(at == AT - 1),
                                tile_position=(0, hl * 64))
                        for hl in range(2):
                            nc.tensor.matmul(
                                pd[hl * 64:hl * 64 + 1, :],
                                lhsT=ones[:],
                                rhs=st[:, (2 * ai + hl) * QB:
                                       (2 * ai + hl + 1) * QB],
                                start=(at == 0), stop=(at == AT - 1),
                                tile_position=(0, hl * 64))
                # evacuate PSUM promptly; normalize downstream on SBUF
                pvs = p_pvs.tile([128, QB], F32, tag="pvs")
                nc.vector.tensor_copy(pvs[:], pv[:])
                rc = p_pvs.tile([65, QB], F32, tag="rcp")
                nc.vector.reciprocal(rc[:], pd[0:65, :])
                scr = p_scr.tile([2, QB], F32, tag="scr")
                nc.sync.dma_start(out=scr[0:1, :], in_=rc[0:1, :])
                nc.sync.dma_start(out=scr[1:2, :], in_=rc[64:65, :])
                rb = p_rcb.tile([128, QB], F32, tag="rb")
                for hl in range(2):
                    nc.sync.dma_start(
                        out=rb[hl * 64:(hl + 1) * 64, :],
                        in_=scr[hl:hl + 1, :].to_broadcast((64, QB)))
                nc.vector.tensor_tensor(
                    out=AOq[qb][:, g, :], in0=pvs[:], in1=rb[:],
                    op=mybir.AluOpType.mult)

            def emit_E(qbi, qi):
                # output rows qt = qbi*4 + qi (128 rows)
                ot = p_out.tile([128, D], F32, tag="ot")
                for dh in range(2):
                    acc = ps_acc.tile([128, 512], F32, tag="acc")
                    for dv in range(DT):
                        nc.tensor.matmul(
                            acc[:],
                            lhsT=AOq[qbi][:, dv, qi * 128:(qi + 1) * 128],
                            rhs=WP[:, dv, dh * 512:(dh + 1) * 512],
                            start=(dv == 0), stop=(dv == DT - 1))
                    nc.vector.tensor_copy(
                        ot[:, dh * 512:(dh + 1) * 512], acc[:])
                qt = qbi * (QB // 128) + qi
                nc.sync.dma_start(
                    out=out[qt * 128:(qt + 1) * 128, :], in_=ot[:])

            # ---------------- software-pipelined schedule ----------------
            # JIT Q-projection: round qb emits C(qb, dq=g+1) one iteration
            # ahead of its own use; only C(qb, 0) crosses the round edge.
            # This keeps the prologue (ACT-idle) to V + KT(0) + C(0,0).
            wvs = [emit_V_panel(0), emit_V_panel(1)]
            xqs = [emit_xq(0)]
            emit_KT(0)
            emit_C_dq(0, 0, xqs[0])
            # skewed (qb=0, g=0): scores+exps interleave with the V
            # projection so ScalarE works during the otherwise-idle
            # prologue. g=0 reads only the vh=0 half of V, so its attn@V
            # follows each V block with a one-tile lag (hides the V
            # PSUM->SBUF copy); the vh=1 V panel is emitted after and is
            # first needed at g=4.
            pv0 = ps_v.tile([128, QB], F32, tag="pv")
            pd0 = ps_d.tile([128, QB], F32, tag="pd")
            sts0 = []
            for at in range(AT):
                sts0.append(emit_scores_exp(0, 0, at))
                emit_V_at(0, wvs[0], at)
                if at > 0:
                    emit_av_denom(0, at - 1, sts0[at - 1], pv0, pd0)
            emit_av_denom(0, AT - 1, sts0[AT - 1], pv0, pd0)
            for at in range(AT):
                emit_V_at(1, wvs[1], at)
            emit_normalize(0, 0, pv0, pd0)
            emit_KT(1)
            emit_C_dq(0, 1, xqs[0])
            for qb in range(NQB):
                if qb + 1 < NQB:
                    xqs.append(emit_xq(qb + 1))
                for g in range(1 if qb == 0 else 0, NPAIR):
                    if qb == 0 and g + 1 < NPAIR:
                        emit_KT(g + 1)
                    emit_attention(qb, g)
                    if g + 1 < NPAIR:
                        emit_C_dq(qb, g + 1, xqs[qb])
                    elif qb + 1 < NQB:
                        emit_C_dq(qb + 1, 0, xqs[qb + 1])
                    if qb >= 1 and g < QB // 128:
                        emit_E(qb - 1, g)
            for qi in range(QB // 128):
                emit_E(NQB - 1, qi)

    nc.compile()
    return nc


def shard_inputs(x, Wqkv, Wq, Wproj, bqkv, bq):
    """Build per-core in_maps (bf16, pre-transposed; biases f32)."""
    bf = ml_dtypes.bfloat16
    xtb = np.ascontiguousarray(np.transpose(x, (0, 2, 1))).astype(bf)  # (B,D,S)
    wq_anchor = np.ascontiguousarray(Wqkv[:, :D]).astype(bf)
    wq_plain = np.ascontiguousarray(Wq).astype(bf)
    wkv_b = np.ascontiguousarray(Wqkv[:, D:3 * D]).astype(bf)
    wpr_b = np.ascontiguousarray(Wproj).astype(bf)
    bq_anchor = np.asarray(bqkv[:D], np.float32)
    bq_plain = np.asarray(bq, np.float32)
    bq2_anchor = np.ascontiguousarray(
        np.stack([bq_anchor, bq_plain], axis=1))
    bq2_plain = np.ascontiguousarray(
        np.stack([bq_plain, bq_plain], axis=1))
    in_maps = []
    for c in range(NCORES):
        b, blk = c // 4, c % 4
        q0 = blk * NQ
        in_maps.append({
            "xT": np.ascontiguousarray(xtb[b, :, q0:q0 + NQ]),
            "aT": np.ascontiguousarray(xtb[b, :, :A]),
            "wq0": wq_anchor if blk == 0 else wq_plain,
            "wq1": wq_plain,
            "wkv": wkv_b,
            "wpr": wpr_b,
            "bq2": bq2_anchor if blk == 0 else bq2_plain,
        })
    return in_maps


def kernel(x, Wqkv, bqkv, Wq, bq, Wproj, bproj, num_anchor_tokens):
    global _cached_nc
    x = np.asarray(x, dtype=np.float32)
    Wqkv = np.asarray(Wqkv, dtype=np.float32)
    Wq = np.asarray(Wq, dtype=np.float32)
    Wproj = np.asarray(Wproj, dtype=np.float32)
    assert int(num_anchor_tokens) == A and x.shape == (B, S, D)

    bqkv = np.asarray(bqkv, dtype=np.float32)
    bq = np.asarray(bq, dtype=np.float32)
    bproj = np.asarray(bproj, dtype=np.float32)
    in_maps = shard_inputs(x, Wqkv, Wq, Wproj, bqkv, bq)
    if _cached_nc is None:
        _cached_nc = build_kernel()

    def run_once():
        res = run_bass_kernel_spmd(_cached_nc, in_maps,
                                   core_ids=list(range(NCORES)))
        o = np.empty((B, S, D), dtype=np.float32)
        for c in range(NCORES):
            b, blk = c // 4, c % 4
            o[b, blk * NQ:(blk + 1) * NQ, :] = res.results[c]["out"]
        # K bias cancels in softmax (constant per-query score shift).
        # V bias adds bv to every attention output -> exact bv@Wproj add.
        o += bqkv[2 * D:3 * D] @ Wproj
        o += bproj
        return o

    # one-row host probe guards against rare transient device corruption
    def probe_err(o):
        r = A  # first non-anchor row of batch 0
        anch = x[0, :A, :]
        Km = anch @ Wqkv[:, D:2 * D] + bqkv[D:2 * D]
        Vm = anch @ Wqkv[:, 2 * D:3 * D] + bqkv[2 * D:3 * D]
        q = x[0, r, :] @ Wq + bq
        row = np.empty(D, np.float32)
        for h in range(H):
            sl = slice(h * HD, (h + 1) * HD)
            s = (Km[:, sl] @ q[sl]) * SCALE
            e = np.exp(s - s.max())
            row[sl] = (e / e.sum()) @ Vm[:, sl]
        ref_row = row @ Wproj + bproj
        return (np.linalg.norm(o[0, r] - ref_row)
                / max(np.linalg.norm(ref_row), 1e-6))

    out = run_once()
    if not np.isfinite(out).all() or probe_err(out) > 5e-2:
        out = run_once()
    return out



# revision 25
# speedup vs baseline: 1.1780x; 1.1780x over previous
"""AnchorAttention distributed Bass kernel for 8 TRN2 NeuronCores.

Problem: x:(2, 8192, 1024) f32; first A=1024 tokens per batch are anchors.
  aqkv = anchors @ Wqkv -> per-head aq, ak, av     (H=16 heads, hd=64)
  qq   = queries @ Wq   -> per-head q
  out  = softmax((concat(aq,qq) @ ak^T)/8) @ av, reshaped, @ Wproj

Sharding: sequence-parallel. Core c in 0..7 owns 2048 token rows:
  batch b = c // 4, rows [2048*(c%4), 2048*(c%4)+2048) of that batch.
Each query row attends only to its batch's anchors, so no collectives are
needed; anchor K/V is recomputed on each core (small). Host pre-transposes
and pre-casts inputs to bf16; rel-err tolerance is ~2e-2 and bf16 compute
lands ~3e-3.

Device kernel layout choices (everything transposed so matmuls chain):
  xT (D, 2048), aT (D, 1024) in DRAM.
  KT[dk, a]   = (anchors @ Wk)^T        via out^T = WkT-tile-stationary mm
  V[a, dv]    = anchors @ Wv            natural
  QT[dq, q]   = (rows @ Wq_eff)^T
  scores^T[a_tile, q] = KT-tile^T @ QT  (K=hd=64; even/odd heads auto
                       row-tile to array rows 0-63 / 64-127 and write
                       different PSUM banks)
  ST = exp(scores * 1/8)  on ScalarE, PSUM->SBUF bf16   (max-free softmax:
                       scores are O(+-8) so exp is safe in f32/bf16)
  attn-out^T[hd, q] accumulated over anchor tiles: lhsT = V tile, the head
                       pair col-packed via tile_position (0,0)/(0,64)
  denom[q] = ones^T @ ST (M=1 matmuls, col-packed to psum partitions 0/64);
  both PSUM accumulators are evacuated immediately (copy / reciprocal) so
  the next pair never stalls on the normalize chain; recips are partition-
  broadcast via a DRAM bounce (stride-0 partition DMA is DRAM-source only)
  and applied as one tensor_tensor multiply into AO.
  final out[q, do]: lhsT = AO tile, rhs = Wproj, f32 DMA out.

  The whole kernel is software-pipelined in 4 q-rounds of 512 columns:
  round qb interleaves attention (ACT-bound: 64 exps of [128,1024]) with
  the NEXT round's Q-projection matmuls and the PREVIOUS round's output-
  projection matmuls, so the PE work of those phases hides under the
  ScalarE exp stream. KT projection rides inside round 0; the V projection
  and first Q-projection block form the prologue.
"""

import sys

if "/opt/trn_rl_repo" not in sys.path:
    sys.path.insert(0, "/opt/trn_rl_repo")

import numpy as np
import ml_dtypes

from concourse import bacc, mybir, tile
from concourse.bass_utils import run_bass_kernel_spmd

# ---------------------------------------------------------------- constants
B, S, D = 2, 8192, 1024
H, HD, A = 16, 64, 1024
NQ = 2048          # token rows per core
NCORES = 8
DT = D // 128      # 8 x 128-row tiles of the model dim
AT = A // 128      # 8 anchor tiles
QB = 512           # q block inside attention
NPAIR = H // 2     # head pairs (adjacent heads share a 128-partition tile)
SCALE = 1.0 / 8.0  # 1/sqrt(hd)

F32 = mybir.dt.float32
BF16 = mybir.dt.bfloat16

_cached_nc = None

BIGEXP = False  # exp over [128, 2048] spanning two anchor tiles (slower)
COLSCORES = False  # scores as M=64-split col-mode mms (no PE mode switches)


def build_kernel(repeat=1):
    nc = bacc.Bacc("TRN2", target_bir_lowering=False, debug=False,
                   num_devices=NCORES)

    xT = nc.declare_dram_parameter("xT", [D, NQ], BF16, isOutput=False)
    aT = nc.declare_dram_parameter("aT", [D, A], BF16, isOutput=False)
    wq0 = nc.declare_dram_parameter("wq0", [D, D], BF16, isOutput=False)
    wq1 = nc.declare_dram_parameter("wq1", [D, D], BF16, isOutput=False)
    wkv = nc.declare_dram_parameter("wkv", [D, 2 * D], BF16, isOutput=False)
    wpr = nc.declare_dram_parameter("wpr", [D, D], BF16, isOutput=False)
    # Q bias, column 0 for q<1024 rows, column 1 for the rest (exact, f32)
    bq2 = nc.declare_dram_parameter("bq2", [D, 2], F32, isOutput=False)
    out = nc.declare_dram_parameter("out", [NQ, D], F32, isOutput=True)

    NQB = NQ // QB  # 4 q rounds

    with tile.TileContext(nc) as tc:
        for _rep in range(repeat):
          with (
            tc.tile_pool(name="attn", bufs=1) as p_attn,      # KT, V, QT
            tc.tile_pool(name="ao", bufs=1) as p_ao,          # AO blocks, WP
            tc.tile_pool(name="stage", bufs=1) as p_stage,    # aT
            tc.tile_pool(name="xq", bufs=2) as p_xq,          # x panel / round
            tc.tile_pool(name="wt", bufs=2) as p_w,           # weight panels
            tc.tile_pool(name="st", bufs=(2 if BIGEXP else 4)) as p_st,
            tc.tile_pool(name="small", bufs=1) as p_small,
            tc.tile_pool(name="pvs", bufs=4) as p_pvs,
            tc.tile_pool(name="rcb", bufs=3) as p_rcb,
            tc.tile_pool(name="scr", bufs=6, space="DRAM") as p_scr,
            tc.tile_pool(name="outsb", bufs=2) as p_out,
            tc.tile_pool(name="psps", bufs=(1 if BIGEXP else 2),
                         space="PSUM") as ps_s,
            tc.tile_pool(name="psacc", bufs=2, space="PSUM") as ps_acc,
            tc.tile_pool(name="psv", bufs=1, space="PSUM") as ps_v,
            tc.tile_pool(name="psd", bufs=1, space="PSUM") as ps_d,
          ):
            KT = p_attn.tile([128, DT, A], BF16, tag="KT")
            V = p_attn.tile([128, AT, D], BF16, tag="V")
            QT = p_attn.tile([128, DT, NQ], BF16, tag="QT")
            AOq = []
            for i in range(NQB):
                ao_i = p_ao.tile([128, DT, QB], BF16, tag=f"AO{i}",
                                 name=f"AO{i}")
                AOq.append(ao_i)
            WP = p_ao.tile([128, DT, D], BF16, tag="WP")
            ones = p_small.tile([128, 1], BF16, tag="ones")
            nc.vector.memset(ones[:], 1.0)
            bqs = p_small.tile([128, DT, 2], F32, tag="bqs")
            nc.sync.dma_start(
                out=bqs[:], in_=bq2[:].rearrange("(k p) c -> p k c", p=128))
            nc.gpsimd.dma_start(
                out=WP[:], in_=wpr[:].rearrange("(k p) c -> p k c", p=128))
            aTs = p_stage.tile([128, DT, A], BF16, tag="aT")
            nc.scalar.dma_start(
                out=aTs[:], in_=aT[:].rearrange("(k p) a -> p k a", p=128))

            # ---------------- emission helpers ---------------------------
            def emit_V_panel(vh):
                wv = p_w.tile([128, DT, 512], BF16, tag="wv", name=f"wv{vh}")
                nc.gpsimd.dma_start(
                    out=wv[:],
                    in_=wkv[:, D + vh * 512:D + (vh + 1) * 512].rearrange(
                        "(k p) c -> p k c", p=128))
                return wv

            def emit_V_at(vh, wv, at):
                acc = ps_acc.tile([128, 512], F32, tag="acc")
                for dn in range(DT):
                    nc.tensor.matmul(
                        acc[:],
                        lhsT=aTs[:, dn, at * 128:(at + 1) * 128],
                        rhs=wv[:, dn, :],
                        start=(dn == 0), stop=(dn == DT - 1))
                nc.vector.tensor_copy(
                    V[:, at, vh * 512:(vh + 1) * 512], acc[:])

            def emit_KT_dma(dk):
                wk = p_w.tile([128, DT, 128], BF16, tag="wk")
                nc.gpsimd.dma_start(
                    out=wk[:],
                    in_=wkv[:, dk * 128:(dk + 1) * 128].rearrange(
                        "(k p) c -> p k c", p=128))
                return wk

            def KT_chunks(dk, wk):
                """KT projection for block dk as 8 filler chunks of 2 mms."""
                state = {}
                chunks = []
                for ah in range(2):
                    for i in range(4):
                        def f(ah=ah, i=i):
                            if i == 0:
                                state[ah] = ps_acc.tile(
                                    [128, 512], F32, tag="acc",
                                    name=f"acc_kt{dk}_{ah}")
                            acc = state[ah]
                            for dn in (2 * i, 2 * i + 1):
                                nc.tensor.matmul(
                                    acc[:],
                                    lhsT=wk[:, dn, :],
                                    rhs=aTs[:, dn, ah * 512:(ah + 1) * 512],
                                    start=(dn == 0), stop=(dn == DT - 1))
                            if i == 3:
                                nc.vector.tensor_copy(
                                    KT[:, dk, ah * 512:(ah + 1) * 512],
                                    acc[:])
                        chunks.append(f)
                return chunks

            def emit_xq(qs):
                xq = p_xq.tile([128, DT, QB], BF16, tag="xq")
                nc.scalar.dma_start(
                    out=xq[:],
                    in_=xT[:, qs * QB:(qs + 1) * QB].rearrange(
                        "(k p) q -> p k q", p=128))
                return xq

            def emit_C_dma(qs, dq):
                half = 0 if qs < 2 else 1
                wsrc = wq0 if half == 0 else wq1
                wq_t = p_w.tile([128, DT, 128], BF16, tag="wq")
                nc.gpsimd.dma_start(
                    out=wq_t[:],
                    in_=wsrc[:, dq * 128:(dq + 1) * 128].rearrange(
                        "(k p) c -> p k c", p=128))
                return wq_t

            def C_chunks(qs, dq, xq, wq_t):
                """Q-projection block (qs, dq) as 4 filler chunks of 2 mms."""
                half = 0 if qs < 2 else 1
                state = {}
                chunks = []
                for i in range(4):
                    def f(i=i):
                        if i == 0:
                            state["acc"] = ps_acc.tile(
                                [128, 512], F32, tag="acc",
                                name=f"acc_c{qs}_{dq}")
                        acc = state["acc"]
                        for dn in (2 * i, 2 * i + 1):
                            nc.tensor.matmul(
                                acc[:],
                                lhsT=wq_t[:, dn, :],
                                rhs=xq[:, dn, :],
                                start=(dn == 0), stop=(dn == DT - 1))
                        if i == 3:
                            nc.vector.tensor_scalar_add(
                                QT[:, dq, qs * QB:(qs + 1) * QB], acc[:],
                                bqs[:, dq, half:half + 1])
                    chunks.append(f)
                return chunks

            def emit_scores_exp(qb, g, at):
                ps = ps_s.tile([128, 2 * QB], F32, tag="ps")
                for hl in range(2):
                    nc.tensor.matmul(
                        ps[:, hl * QB:(hl + 1) * QB],
                        lhsT=KT[hl * 64:(hl + 1) * 64, g,
                                at * 128:(at + 1) * 128],
                        rhs=QT[hl * 64:(hl + 1) * 64, g,
                               qb * QB:(qb + 1) * QB],
                        start=True, stop=True)
                st = p_st.tile([128, 2 * QB], BF16, tag="st")
                nc.scalar.activation(
                    st[:], ps[:], mybir.ActivationFunctionType.Exp,
                    scale=SCALE)
                return st

            def emit_av_denom(g, at, st, pv, pd):
                for hl in range(2):
                    h = 2 * g + hl
                    nc.tensor.matmul(
                        pv[hl * 64:(hl + 1) * 64, :],
                        lhsT=V[:, at, h * 64:(h + 1) * 64],
                        rhs=st[:, hl * QB:(hl + 1) * QB],
                        start=(at == 0), stop=(at == AT - 1),
                        tile_position=(0, hl * 64))
                for hl in range(2):
                    nc.tensor.matmul(
                        pd[hl * 64:hl * 64 + 1, :],
                        lhsT=ones[:],
                        rhs=st[:, hl * QB:(hl + 1) * QB],
                        start=(at == 0), stop=(at == AT - 1),
                        tile_position=(0, hl * 64))

            def emit_normalize(qb, g, pv, pd):
                pvs = p_pvs.tile([128, QB], F32, tag="pvs")
                nc.vector.tensor_copy(pvs[:], pv[:])
                rc = p_pvs.tile([65, QB], F32, tag="rcp")
                nc.vector.reciprocal(rc[:], pd[0:65, :])
                scr = p_scr.tile([2, QB], F32, tag="scr")
                nc.sync.dma_start(out=scr[0:1, :], in_=rc[0:1, :])
                nc.sync.dma_start(out=scr[1:2, :], in_=rc[64:65, :])
                rb = p_rcb.tile([128, QB], F32, tag="rb")
                for hl in range(2):
                    nc.sync.dma_start(
                        out=rb[hl * 64:(hl + 1) * 64, :],
                        in_=scr[hl:hl + 1, :].to_broadcast((64, QB)))
                nc.vector.tensor_tensor(
                    out=AOq[qb][:, g, :], in0=pvs[:], in1=rb[:],
                    op=mybir.AluOpType.mult)

            def emit_attention(qb, g, fillers=(), at_hook=None):
                """One head-pair attention step, fillers interleaved.

                Per at-slot: scores pair (row-tiled, concurrent), then the
                LAGGED av+denom for at-1 (so it never waits on ScalarE's
                exp), then a slice of the filler closures (projection /
                KT / V matmul chunks) to soak the PE while ScalarE chews
                through the exp stream.
                """
                pv = ps_v.tile([128, QB], F32, tag="pv")
                pd = ps_d.tile([128, QB], F32, tag="pd")
                fillers = list(fillers)
                nf = len(fillers)
                fi = 0
                sts = []
                for at in range(AT):
                    sts.append(emit_scores_exp(qb, g, at))
                    if at_hook is not None:
                        at_hook(at)
                    if at > 0:
                        emit_av_denom(g, at - 1, sts[at - 1], pv, pd)
                    tgt = (at + 1) * nf // AT
                    while fi < tgt:
                        fillers[fi]()
                        fi += 1
                emit_av_denom(g, AT - 1, sts[AT - 1], pv, pd)
                emit_normalize(qb, g, pv, pd)

            def E_chunks(qbi, qi):
                """Out-projection of tile (qbi, qi) as 8 chunks of 2 mms."""
                state = {}
                chunks = []
                for dh in range(2):
                    for i in range(4):
                        def f(dh=dh, i=i):
                            if dh == 0 and i == 0:
                                state["ot"] = p_out.tile(
                                    [128, D], F32, tag="ot",
                                    name=f"ot{qbi}_{qi}")
                            if i == 0:
                                state["acc"] = ps_acc.tile(
                                    [128, 512], F32, tag="acc",
                                    name=f"acc_e{qbi}_{qi}_{dh}")
                            acc = state["acc"]
                            for dv in (2 * i, 2 * i + 1):
                                nc.tensor.matmul(
                                    acc[:],
                                    lhsT=AOq[qbi][:, dv,
                                                  qi * 128:(qi + 1) * 128],
                                    rhs=WP[:, dv, dh * 512:(dh + 1) * 512],
                                    start=(dv == 0), stop=(dv == DT - 1))
                            if i == 3:
                                nc.vector.tensor_copy(
                                    state["ot"][:, dh * 512:(dh + 1) * 512],
                                    acc[:])
                                if dh == 1:
                                    qt = qbi * (QB // 128) + qi
                                    nc.sync.dma_start(
                                        out=out[qt * 128:(qt + 1) * 128, :],
                                        in_=state["ot"][:])
                        chunks.append(f)
                return chunks

            # ---------------- software-pipelined schedule ----------------
            # Every g-step interleaves its attention at-loop with filler
            # chunks: the JIT Q-projection C(qb, g+1), round-0's KT(g+1)
            # and vh=1 V blocks, and rounds>=1's out-projection of the
            # previous round.  Weight DMAs for a step's fillers are issued
            # one full g-step ahead so the chunks never wait on HBM.
            def c_target(qb, g):
                if g + 1 < NPAIR:
                    return (qb, g + 1)
                if qb + 1 < NQB:
                    return (qb + 1, 0)
                return None

            wvs = [emit_V_panel(0), emit_V_panel(1)]
            xqs = [emit_xq(0)]
            wkts = {0: emit_KT_dma(0)}
            wqts = {(0, 0): emit_C_dma(0, 0)}
            for f in KT_chunks(0, wkts[0]):
                f()
            for f in C_chunks(0, 0, xqs[0], wqts[(0, 0)]):
                f()
            wkts[1] = emit_KT_dma(1)
            wqts[(0, 1)] = emit_C_dma(0, 1)
            # vh=1 V blocks are first needed at g=4; spread them over g=1..3
            v1_assign = {1: (0, 1, 2), 2: (3, 4, 5), 3: (6, 7)}
            e_rr = []
            for qb in range(NQB):
                if qb + 1 < NQB:
                    xqs.append(emit_xq(qb + 1))
                if qb >= 1:
                    e_rr = [c for qi in range(QB // 128)
                            for c in E_chunks(qb - 1, qi)]
                for g in range(NPAIR):
                    # prefetch weight panels for the NEXT step's fillers
                    ns = (qb, g + 1) if g + 1 < NPAIR else (
                        (qb + 1, 0) if qb + 1 < NQB else None)
                    if ns is not None:
                        t2 = c_target(*ns)
                        if t2 is not None and t2 not in wqts:
                            wqts[t2] = emit_C_dma(*t2)
                        if ns[0] == 0 and ns[1] + 1 < NPAIR \
                                and (ns[1] + 1) not in wkts:
                            wkts[ns[1] + 1] = emit_KT_dma(ns[1] + 1)
                    # fillers for THIS step (C first: it is needed by g+1)
                    fillers = []
                    t = c_target(qb, g)
                    if t is not None:
                        fillers += C_chunks(t[0], t[1], xqs[t[0]], wqts[t])
                    if qb == 0 and g + 1 < NPAIR:
                        fillers += KT_chunks(g + 1, wkts[g + 1])
                    if qb == 0 and g in v1_assign:
                        for at_ in v1_assign[g]:
                            fillers.append(
                                lambda at_=at_: emit_V_at(1, wvs[1], at_))
                    if qb >= 1:
                        fillers += e_rr[g * 4:(g + 1) * 4]
                    at_hook = None
                    if qb == 0 and g == 0:
                        at_hook = lambda at: emit_V_at(0, wvs[0], at)
                    emit_attention(qb, g, fillers, at_hook)
            for qi in range(QB // 128):
                for f in E_chunks(NQB - 1, qi):
                    f()

    nc.compile()
    return nc


def shard_inputs(x, Wqkv, Wq, Wproj, bqkv, bq):
    """Build per-core in_maps (bf16, pre-transposed; biases f32)."""
    bf = ml_dtypes.bfloat16
    xtb = np.ascontiguousarray(np.transpose(x, (0, 2, 1))).astype(bf)  # (B,D,S)
    wq_anchor = np.ascontiguousarray(Wqkv[:, :D]).astype(bf)
    wq_plain = np.ascontiguousarray(Wq).astype(bf)
    wkv_b = np.ascontiguousarray(Wqkv[:, D:3 * D]).astype(bf)
    wpr_b = np.ascontiguousarray(Wproj).astype(bf)
    bq_anchor = np.asarray(bqkv[:D], np.float32)
    bq_plain = np.asarray(bq, np.float32)
    bq2_anchor = np.ascontiguousarray(
        np.stack([bq_anchor, bq_plain], axis=1))
    bq2_plain = np.ascontiguousarray(
        np.stack([bq_plain, bq_plain], axis=1))
    in_maps = []
    for c in range(NCORES):
        b, blk = c // 4, c % 4
        q0 = blk * NQ
        in_maps.append({
            "xT": np.ascontiguousarray(xtb[b, :, q0:q0 + NQ]),
            "aT": np.ascontiguousarray(xtb[b, :, :A]),
            "wq0": wq_anchor if blk == 0 else wq_plain,
            "wq1": wq_plain,
            "wkv": wkv_b,
            "wpr": wpr_b,
            "bq2": bq2_anchor if blk == 0 else bq2_plain,
        })
    return in_maps


def kernel(x, Wqkv, bqkv, Wq, bq, Wproj, bproj, num_anchor_tokens):
    global _cached_nc
    x = np.asarray(x, dtype=np.float32)
    Wqkv = np.asarray(Wqkv, dtype=np.float32)
    Wq = np.asarray(Wq, dtype=np.float32)
    Wproj = np.asarray(Wproj, dtype=np.float32)
    assert int(num_anchor_tokens) == A and x.shape == (B, S, D)

    bqkv = np.asarray(bqkv, dtype=np.float32)
    bq = np.asarray(bq, dtype=np.float32)
    bproj = np.asarray(bproj, dtype=np.float32)
    in_maps = shard_inputs(x, Wqkv, Wq, Wproj, bqkv, bq)
    if _cached_nc is None:
        _cached_nc = build_kernel()

    def run_once():
        res = run_bass_kernel_spmd(_cached_nc, in_maps,
                                   core_ids=list(range(NCORES)))
        o = np.empty((B, S, D), dtype=np.float32)
        for c in range(NCORES):
            b, blk = c // 4, c % 4
            o[b, blk * NQ:(blk + 1) * NQ, :] = res.results[c]["out"]
        # K bias cancels in softmax (constant per-query score shift).
        # V bias adds bv to every attention output -> exact bv@Wproj add.
        o += bqkv[2 * D:3 * D] @ Wproj
        o += bproj
        return o

    # one-row host probe guards against rare transient device corruption
    def probe_err(o):
        r = A  # first non-anchor row of batch 0
        anch = x[0, :A, :]
        Km = anch @ Wqkv[:, D:2 * D] + bqkv[D:2 * D]
        Vm = anch @ Wqkv[:, 2 * D:3 * D] + bqkv[2 * D:3 * D]
        q = x[0, r, :] @ Wq + bq
        row = np.empty(D, np.float32)
        for h in range(H):
            sl = slice(h * HD, (h + 1) * HD)
            s = (Km[:, sl] @ q[sl]) * SCALE
            e = np.exp(s - s.max())
            row[sl] = (e / e.sum()) @ Vm[:, sl]
        ref_row = row @ Wproj + bproj
        return (np.linalg.norm(o[0, r] - ref_row)
                / max(np.linalg.norm(ref_row), 1e-6))

    out = run_once()
    if not np.isfinite(out).all() or probe_err(out) > 5e-2:
        out = run_once()
    return out



# revision 26
# speedup vs baseline: 1.3267x; 1.1263x over previous
"""AnchorAttention distributed Bass kernel for 8 TRN2 NeuronCores.

Problem: x:(2, 8192, 1024) f32; first A=1024 tokens per batch are anchors.
  aqkv = anchors @ Wqkv -> per-head aq, ak, av     (H=16 heads, hd=64)
  qq   = queries @ Wq   -> per-head q
  out  = softmax((concat(aq,qq) @ ak^T)/8) @ av, reshaped, @ Wproj

Sharding: sequence-parallel. Core c in 0..7 owns 2048 token rows:
  batch b = c // 4, rows [2048*(c%4), 2048*(c%4)+2048) of that batch.
Each query row attends only to its batch's anchors, so no collectives are
needed; anchor K/V is recomputed on each core (small). Host pre-transposes
and pre-casts inputs to bf16; rel-err tolerance is ~2e-2 and bf16 compute
lands ~3e-3.

Device kernel layout choices (everything transposed so matmuls chain):
  xT (D, 2048), aT (D, 1024) in DRAM.
  KT[dk, a]   = (anchors @ Wk)^T        via out^T = WkT-tile-stationary mm
  V[a, dv]    = anchors @ Wv            natural
  QT[dq, q]   = (rows @ Wq_eff)^T
  scores^T[a_tile, q] = KT-tile^T @ QT  (K=hd=64; even/odd heads auto
                       row-tile to array rows 0-63 / 64-127 and write
                       different PSUM banks)
  ST = exp(scores * 1/8)  on ScalarE, PSUM->SBUF bf16   (max-free softmax:
                       scores are O(+-8) so exp is safe in f32/bf16)
  attn-out^T[hd, q] accumulated over anchor tiles: lhsT = V tile, the head
                       pair col-packed via tile_position (0,0)/(0,64)
  denom[q] = ones^T @ ST (M=1 matmuls, col-packed to psum partitions 0/64);
  both PSUM accumulators are evacuated immediately (copy / reciprocal) so
  the next pair never stalls on the normalize chain; recips are partition-
  broadcast via a DRAM bounce (stride-0 partition DMA is DRAM-source only)
  and applied as one tensor_tensor multiply into AO.
  final out[q, do]: lhsT = AO tile, rhs = Wproj, f32 DMA out.

  The whole kernel is software-pipelined in 4 q-rounds of 512 columns:
  round qb interleaves attention (ACT-bound: 64 exps of [128,1024]) with
  the NEXT round's Q-projection matmuls and the PREVIOUS round's output-
  projection matmuls, so the PE work of those phases hides under the
  ScalarE exp stream. KT projection rides inside round 0; the V projection
  and first Q-projection block form the prologue.
"""

import sys

if "/opt/trn_rl_repo" not in sys.path:
    sys.path.insert(0, "/opt/trn_rl_repo")

import numpy as np
import ml_dtypes

from concourse import bacc, mybir, tile
from concourse.bass_utils import run_bass_kernel_spmd

# ---------------------------------------------------------------- constants
B, S, D = 2, 8192, 1024
H, HD, A = 16, 64, 1024
NQ = 2048          # token rows per core
NCORES = 8
DT = D // 128      # 8 x 128-row tiles of the model dim
AT = A // 128      # 8 anchor tiles
QB = 512           # q block inside attention
NPAIR = H // 2     # head pairs (adjacent heads share a 128-partition tile)
SCALE = 1.0 / 8.0  # 1/sqrt(hd)

F32 = mybir.dt.float32
BF16 = mybir.dt.bfloat16

_cached_nc = None

BIGEXP = False  # exp over [128, 2048] spanning two anchor tiles (slower)
COLSCORES = False  # scores as M=64-split col-mode mms (no PE mode switches)


def build_kernel(repeat=1):
    nc = bacc.Bacc("TRN2", target_bir_lowering=False, debug=False,
                   num_devices=NCORES)

    xT = nc.declare_dram_parameter("xT", [D, NQ], BF16, isOutput=False)
    aT = nc.declare_dram_parameter("aT", [D, A], BF16, isOutput=False)
    wq0 = nc.declare_dram_parameter("wq0", [D, D], BF16, isOutput=False)
    wq1 = nc.declare_dram_parameter("wq1", [D, D], BF16, isOutput=False)
    wkv = nc.declare_dram_parameter("wkv", [D, 2 * D], BF16, isOutput=False)
    wpr = nc.declare_dram_parameter("wpr", [D, D], BF16, isOutput=False)
    # Q bias, column 0 for q<1024 rows, column 1 for the rest (exact, f32)
    bq2 = nc.declare_dram_parameter("bq2", [D, 2], F32, isOutput=False)
    out = nc.declare_dram_parameter("out", [NQ, D], F32, isOutput=True)

    NQB = NQ // QB  # 4 q rounds

    with tile.TileContext(nc) as tc:
        for _rep in range(repeat):
          with (
            tc.tile_pool(name="attn", bufs=1) as p_attn,      # KT, V, QT
            tc.tile_pool(name="ao", bufs=1) as p_ao,          # AO blocks, WP
            tc.tile_pool(name="stage", bufs=1) as p_stage,    # aT
            tc.tile_pool(name="xq", bufs=2) as p_xq,          # x panel / round
            tc.tile_pool(name="wt", bufs=2) as p_w,           # weight panels
            tc.tile_pool(name="st", bufs=(2 if BIGEXP else 4)) as p_st,
            tc.tile_pool(name="small", bufs=1) as p_small,
            tc.tile_pool(name="pvs", bufs=4) as p_pvs,
            tc.tile_pool(name="rcb", bufs=3) as p_rcb,
            tc.tile_pool(name="scr", bufs=6, space="DRAM") as p_scr,
            tc.tile_pool(name="outsb", bufs=2) as p_out,
            tc.tile_pool(name="psps", bufs=(1 if BIGEXP else 2),
                         space="PSUM") as ps_s,
            tc.tile_pool(name="psacc", bufs=2, space="PSUM") as ps_acc,
            tc.tile_pool(name="psv", bufs=1, space="PSUM") as ps_v,
            tc.tile_pool(name="psd", bufs=1, space="PSUM") as ps_d,
          ):
            KT = p_attn.tile([128, DT, A], BF16, tag="KT")
            V = p_attn.tile([128, AT, D], BF16, tag="V")
            QT = p_attn.tile([128, DT, NQ], BF16, tag="QT")
            AOq = []
            for i in range(NQB):
                ao_i = p_ao.tile([128, DT, QB], BF16, tag=f"AO{i}",
                                 name=f"AO{i}")
                AOq.append(ao_i)
            WP = p_ao.tile([128, DT, D], BF16, tag="WP")
            ones = p_small.tile([128, 1], BF16, tag="ones")
            nc.vector.memset(ones[:], 1.0)
            bqs = p_small.tile([128, DT, 2], F32, tag="bqs")
            nc.sync.dma_start(
                out=bqs[:], in_=bq2[:].rearrange("(k p) c -> p k c", p=128))
            nc.gpsimd.dma_start(
                out=WP[:], in_=wpr[:].rearrange("(k p) c -> p k c", p=128))
            aTs = p_stage.tile([128, DT, A], BF16, tag="aT")
            nc.scalar.dma_start(
                out=aTs[:], in_=aT[:].rearrange("(k p) a -> p k a", p=128))

            # ---------------- emission helpers ---------------------------
            def emit_V_panel(vh):
                wv = p_w.tile([128, DT, 512], BF16, tag="wv", name=f"wv{vh}")
                nc.gpsimd.dma_start(
                    out=wv[:],
                    in_=wkv[:, D + vh * 512:D + (vh + 1) * 512].rearrange(
                        "(k p) c -> p k c", p=128))
                return wv

            def emit_V_at(vh, wv, at):
                acc = ps_acc.tile([128, 512], F32, tag="acc")
                for dn in range(DT):
                    nc.tensor.matmul(
                        acc[:],
                        lhsT=aTs[:, dn, at * 128:(at + 1) * 128],
                        rhs=wv[:, dn, :],
                        start=(dn == 0), stop=(dn == DT - 1))
                nc.vector.tensor_copy(
                    V[:, at, vh * 512:(vh + 1) * 512], acc[:])

            def emit_KT_dma(dk):
                wk = p_w.tile([128, DT, 128], BF16, tag="wk")
                nc.gpsimd.dma_start(
                    out=wk[:],
                    in_=wkv[:, dk * 128:(dk + 1) * 128].rearrange(
                        "(k p) c -> p k c", p=128))
                return wk

            def KT_chunks(dk, wk):
                """KT projection for block dk as 8 filler chunks of 2 mms."""
                state = {}
                chunks = []
                for ah in range(2):
                    for i in range(4):
                        def f(ah=ah, i=i):
                            if i == 0:
                                state[ah] = ps_acc.tile(
                                    [128, 512], F32, tag="acc",
                                    name=f"acc_kt{dk}_{ah}")
                            acc = state[ah]
                            for dn in (2 * i, 2 * i + 1):
                                nc.tensor.matmul(
                                    acc[:],
                                    lhsT=wk[:, dn, :],
                                    rhs=aTs[:, dn, ah * 512:(ah + 1) * 512],
                                    start=(dn == 0), stop=(dn == DT - 1))
                            if i == 3:
                                nc.vector.tensor_copy(
                                    KT[:, dk, ah * 512:(ah + 1) * 512],
                                    acc[:])
                        chunks.append(f)
                return chunks

            def emit_xq(qs):
                xq = p_xq.tile([128, DT, QB], BF16, tag="xq")
                nc.scalar.dma_start(
                    out=xq[:],
                    in_=xT[:, qs * QB:(qs + 1) * QB].rearrange(
                        "(k p) q -> p k q", p=128))
                return xq

            def emit_C_dma(qs, dq):
                half = 0 if qs < 2 else 1
                wsrc = wq0 if half == 0 else wq1
                wq_t = p_w.tile([128, DT, 128], BF16, tag="wq")
                nc.gpsimd.dma_start(
                    out=wq_t[:],
                    in_=wsrc[:, dq * 128:(dq + 1) * 128].rearrange(
                        "(k p) c -> p k c", p=128))
                return wq_t

            def C_chunks(qs, dq, xq, wq_t):
                """Q-projection block (qs, dq) as 4 filler chunks of 2 mms."""
                half = 0 if qs < 2 else 1
                state = {}
                chunks = []
                for i in range(4):
                    def f(i=i):
                        if i == 0:
                            state["acc"] = ps_acc.tile(
                                [128, 512], F32, tag="acc",
                                name=f"acc_c{qs}_{dq}")
                        acc = state["acc"]
                        for dn in (2 * i, 2 * i + 1):
                            nc.tensor.matmul(
                                acc[:],
                                lhsT=wq_t[:, dn, :],
                                rhs=xq[:, dn, :],
                                start=(dn == 0), stop=(dn == DT - 1))
                        if i == 3:
                            nc.vector.tensor_scalar_add(
                                QT[:, dq, qs * QB:(qs + 1) * QB], acc[:],
                                bqs[:, dq, half:half + 1])
                    chunks.append(f)
                return chunks

            def emit_scores_exp(qb, g, at):
                ps = ps_s.tile([128, 2 * QB], F32, tag="ps")
                for hl in range(2):
                    nc.tensor.matmul(
                        ps[:, hl * QB:(hl + 1) * QB],
                        lhsT=KT[hl * 64:(hl + 1) * 64, g,
                                at * 128:(at + 1) * 128],
                        rhs=QT[hl * 64:(hl + 1) * 64, g,
                               qb * QB:(qb + 1) * QB],
                        start=True, stop=True)
                st = p_st.tile([128, 2 * QB], BF16, tag="st")
                nc.scalar.activation(
                    st[:], ps[:], mybir.ActivationFunctionType.Exp,
                    scale=SCALE)
                return st

            def emit_av_denom(g, at, st, pv, pd):
                for hl in range(2):
                    h = 2 * g + hl
                    nc.tensor.matmul(
                        pv[hl * 64:(hl + 1) * 64, :],
                        lhsT=V[:, at, h * 64:(h + 1) * 64],
                        rhs=st[:, hl * QB:(hl + 1) * QB],
                        start=(at == 0), stop=(at == AT - 1),
                        tile_position=(0, hl * 64))
                for hl in range(2):
                    nc.tensor.matmul(
                        pd[hl * 64:hl * 64 + 1, :],
                        lhsT=ones[:],
                        rhs=st[:, hl * QB:(hl + 1) * QB],
                        start=(at == 0), stop=(at == AT - 1),
                        tile_position=(0, hl * 64))

            def emit_normalize(qb, g, pv, pd):
                pvs = p_pvs.tile([128, QB], F32, tag="pvs")
                nc.vector.tensor_copy(pvs[:], pv[:])
                rc = p_pvs.tile([65, QB], F32, tag="rcp")
                nc.vector.reciprocal(rc[:], pd[0:65, :])
                scr = p_scr.tile([2, QB], F32, tag="scr")
                nc.sync.dma_start(out=scr[0:1, :], in_=rc[0:1, :])
                nc.sync.dma_start(out=scr[1:2, :], in_=rc[64:65, :])
                rb = p_rcb.tile([128, QB], F32, tag="rb")
                for hl in range(2):
                    nc.sync.dma_start(
                        out=rb[hl * 64:(hl + 1) * 64, :],
                        in_=scr[hl:hl + 1, :].to_broadcast((64, QB)))
                nc.vector.tensor_tensor(
                    out=AOq[qb][:, g, :], in0=pvs[:], in1=rb[:],
                    op=mybir.AluOpType.mult)

            def emit_attention(qb, g, fillers=(), at_hook=None):
                """One head-pair attention step, fillers interleaved.

                Per at-slot: scores pair (row-tiled, concurrent), then the
                LAGGED av+denom for at-1 (so it never waits on ScalarE's
                exp), then a slice of the filler closures (projection /
                KT / V matmul chunks) to soak the PE while ScalarE chews
                through the exp stream.
                """
                pv = ps_v.tile([128, QB], F32, tag="pv")
                pd = ps_d.tile([128, QB], F32, tag="pd")
                fillers = list(fillers)
                nf = len(fillers)
                fi = 0
                sts = []
                for at in range(AT):
                    sts.append(emit_scores_exp(qb, g, at))
                    if at_hook is not None:
                        at_hook(at)
                    if at > 0:
                        emit_av_denom(g, at - 1, sts[at - 1], pv, pd)
                    tgt = (at + 1) * nf // AT
                    while fi < tgt:
                        fillers[fi]()
                        fi += 1
                emit_av_denom(g, AT - 1, sts[AT - 1], pv, pd)
                emit_normalize(qb, g, pv, pd)

            def E_chunks(qbi, qi):
                """Out-projection of tile (qbi, qi) as 8 chunks of 2 mms."""
                state = {}
                chunks = []
                for dh in range(2):
                    for i in range(4):
                        def f(dh=dh, i=i):
                            if dh == 0 and i == 0:
                                state["ot"] = p_out.tile(
                                    [128, D], F32, tag="ot",
                                    name=f"ot{qbi}_{qi}")
                            if i == 0:
                                state["acc"] = ps_acc.tile(
                                    [128, 512], F32, tag="acc",
                                    name=f"acc_e{qbi}_{qi}_{dh}")
                            acc = state["acc"]
                            for dv in (2 * i, 2 * i + 1):
                                nc.tensor.matmul(
                                    acc[:],
                                    lhsT=AOq[qbi][:, dv,
                                                  qi * 128:(qi + 1) * 128],
                                    rhs=WP[:, dv, dh * 512:(dh + 1) * 512],
                                    start=(dv == 0), stop=(dv == DT - 1))
                            if i == 3:
                                nc.vector.tensor_copy(
                                    state["ot"][:, dh * 512:(dh + 1) * 512],
                                    acc[:])
                                if dh == 1:
                                    qt = qbi * (QB // 128) + qi
                                    nc.sync.dma_start(
                                        out=out[qt * 128:(qt + 1) * 128, :],
                                        in_=state["ot"][:])
                        chunks.append(f)
                return chunks

            # ---------------- software-pipelined schedule ----------------
            # Every g-step interleaves its attention at-loop with filler
            # chunks: the JIT Q-projection C(qb, g+1), round-0's KT(g+1)
            # and vh=1 V blocks, and rounds>=1's out-projection of the
            # previous round.  Weight DMAs for a step's fillers are issued
            # one full g-step ahead so the chunks never wait on HBM.
            def c_target(qb, g):
                """C block computed during step (qb, g): two steps ahead,
                so the QT write never races the consuming scores."""
                s = qb * NPAIR + g + 2
                if s < NQB * NPAIR:
                    return (s // NPAIR, s % NPAIR)
                return None

            wvs = [emit_V_panel(0), emit_V_panel(1)]
            xqs = [emit_xq(0)]
            wkts = {0: emit_KT_dma(0)}
            wqts = {(0, 0): emit_C_dma(0, 0), (0, 1): emit_C_dma(0, 1)}
            for f in KT_chunks(0, wkts[0]):
                f()
            for f in C_chunks(0, 0, xqs[0], wqts[(0, 0)]):
                f()
            for f in C_chunks(0, 1, xqs[0], wqts[(0, 1)]):
                f()
            wkts[1] = emit_KT_dma(1)
            wqts[(0, 2)] = emit_C_dma(0, 2)
            # vh=1 V blocks are first needed at g=4; spread them over g=1..3
            v1_assign = {1: (0, 1, 2), 2: (3, 4, 5), 3: (6, 7)}
            e_rr = []
            for qb in range(NQB):
                if qb + 1 < NQB:
                    xqs.append(emit_xq(qb + 1))
                if qb >= 1:
                    e_rr = [c for qi in range(QB // 128)
                            for c in E_chunks(qb - 1, qi)]
                for g in range(NPAIR):
                    # prefetch weight panels for the NEXT step's fillers
                    ns = (qb, g + 1) if g + 1 < NPAIR else (
                        (qb + 1, 0) if qb + 1 < NQB else None)
                    if ns is not None:
                        t2 = c_target(*ns)
                        if t2 is not None and t2 not in wqts:
                            wqts[t2] = emit_C_dma(*t2)
                        if ns[0] == 0 and ns[1] + 1 < NPAIR \
                                and (ns[1] + 1) not in wkts:
                            wkts[ns[1] + 1] = emit_KT_dma(ns[1] + 1)
                    # fillers for THIS step (C first: it is needed by g+1)
                    fillers = []
                    t = c_target(qb, g)
                    if t is not None:
                        fillers += C_chunks(t[0], t[1], xqs[t[0]], wqts[t])
                    if qb == 0 and g + 1 < NPAIR:
                        fillers += KT_chunks(g + 1, wkts[g + 1])
                    if qb == 0 and g in v1_assign:
                        for at_ in v1_assign[g]:
                            fillers.append(
                                lambda at_=at_: emit_V_at(1, wvs[1], at_))
                    if qb >= 1:
                        fillers += e_rr[g * 4:(g + 1) * 4]
                    at_hook = None
                    if qb == 0 and g == 0:
                        at_hook = lambda at: emit_V_at(0, wvs[0], at)
                    emit_attention(qb, g, fillers, at_hook)
            for qi in range(QB // 128):
                for f in E_chunks(NQB - 1, qi):
                    f()

    nc.compile()
    return nc


def shard_inputs(x, Wqkv, Wq, Wproj, bqkv, bq):
    """Build per-core in_maps (bf16, pre-transposed; biases f32)."""
    bf = ml_dtypes.bfloat16
    xtb = np.ascontiguousarray(np.transpose(x, (0, 2, 1))).astype(bf)  # (B,D,S)
    wq_anchor = np.ascontiguousarray(Wqkv[:, :D]).astype(bf)
    wq_plain = np.ascontiguousarray(Wq).astype(bf)
    wkv_b = np.ascontiguousarray(Wqkv[:, D:3 * D]).astype(bf)
    wpr_b = np.ascontiguousarray(Wproj).astype(bf)
    bq_anchor = np.asarray(bqkv[:D], np.float32)
    bq_plain = np.asarray(bq, np.float32)
    bq2_anchor = np.ascontiguousarray(
        np.stack([bq_anchor, bq_plain], axis=1))
    bq2_plain = np.ascontiguousarray(
        np.stack([bq_plain, bq_plain], axis=1))
    in_maps = []
    for c in range(NCORES):
        b, blk = c // 4, c % 4
        q0 = blk * NQ
        in_maps.append({
            "xT": np.ascontiguousarray(xtb[b, :, q0:q0 + NQ]),
            "aT": np.ascontiguousarray(xtb[b, :, :A]),
            "wq0": wq_anchor if blk == 0 else wq_plain,
            "wq1": wq_plain,
            "wkv": wkv_b,
            "wpr": wpr_b,
            "bq2": bq2_anchor if blk == 0 else bq2_plain,
        })
    return in_maps


def kernel(x, Wqkv, bqkv, Wq, bq, Wproj, bproj, num_anchor_tokens):
    global _cached_nc
    x = np.asarray(x, dtype=np.float32)
    Wqkv = np.asarray(Wqkv, dtype=np.float32)
    Wq = np.asarray(Wq, dtype=np.float32)
    Wproj = np.asarray(Wproj, dtype=np.float32)
    assert int(num_anchor_tokens) == A and x.shape == (B, S, D)

    bqkv = np.asarray(bqkv, dtype=np.float32)
    bq = np.asarray(bq, dtype=np.float32)
    bproj = np.asarray(bproj, dtype=np.float32)
    in_maps = shard_inputs(x, Wqkv, Wq, Wproj, bqkv, bq)
    if _cached_nc is None:
        _cached_nc = build_kernel()

    def run_once():
        res = run_bass_kernel_spmd(_cached_nc, in_maps,
                                   core_ids=list(range(NCORES)))
        o = np.empty((B, S, D), dtype=np.float32)
        for c in range(NCORES):
            b, blk = c // 4, c % 4
            o[b, blk * NQ:(blk + 1) * NQ, :] = res.results[c]["out"]
        # K bias cancels in softmax (constant per-query score shift).
        # V bias adds bv to every attention output -> exact bv@Wproj add.
        o += bqkv[2 * D:3 * D] @ Wproj
        o += bproj
        return o

    # one-row host probe guards against rare transient device corruption
    def probe_err(o):
        r = A  # first non-anchor row of batch 0
        anch = x[0, :A, :]
        Km = anch @ Wqkv[:, D:2 * D] + bqkv[D:2 * D]
        Vm = anch @ Wqkv[:, 2 * D:3 * D] + bqkv[2 * D:3 * D]
        q = x[0, r, :] @ Wq + bq
        row = np.empty(D, np.float32)
        for h in range(H):
            sl = slice(h * HD, (h + 1) * HD)
            s = (Km[:, sl] @ q[sl]) * SCALE
            e = np.exp(s - s.max())
            row[sl] = (e / e.sum()) @ Vm[:, sl]
        ref_row = row @ Wproj + bproj
        return (np.linalg.norm(o[0, r] - ref_row)
                / max(np.linalg.norm(ref_row), 1e-6))

    out = run_once()
    if not np.isfinite(out).all() or probe_err(out) > 5e-2:
        out = run_once()
    return out



# revision 27
# speedup vs baseline: 1.5045x; 1.1340x over previous
"""AnchorAttention distributed Bass kernel for 8 TRN2 NeuronCores.

Problem: x:(2, 8192, 1024) f32; first A=1024 tokens per batch are anchors.
  aqkv = anchors @ Wqkv -> per-head aq, ak, av     (H=16 heads, hd=64)
  qq   = queries @ Wq   -> per-head q
  out  = softmax((concat(aq,qq) @ ak^T)/8) @ av, reshaped, @ Wproj

Sharding: sequence-parallel. Core c in 0..7 owns 2048 token rows:
  batch b = c // 4, rows [2048*(c%4), 2048*(c%4)+2048) of that batch.
Each query row attends only to its batch's anchors, so no collectives are
needed; anchor K/V is recomputed on each core (small). Host pre-transposes
and pre-casts inputs to bf16; rel-err tolerance is ~2e-2 and bf16 compute
lands ~3e-3.

Device kernel layout choices (everything transposed so matmuls chain):
  xT (D, 2048), aT (D, 1024) in DRAM.
  KT[dk, a]   = (anchors @ Wk)^T        via out^T = WkT-tile-stationary mm
  V[a, dv]    = anchors @ Wv            natural
  QT[dq, q]   = (rows @ Wq_eff)^T
  scores^T[a_tile, q] = KT-tile^T @ QT  (K=hd=64; even/odd heads auto
                       row-tile to array rows 0-63 / 64-127 and write
                       different PSUM banks)
  ST = exp(scores * 1/8)  on ScalarE, PSUM->SBUF bf16   (max-free softmax:
                       scores are O(+-8) so exp is safe in f32/bf16)
  attn-out^T[hd, q] accumulated over anchor tiles: lhsT = V tile, the head
                       pair col-packed via tile_position (0,0)/(0,64)
  denom[q] = ones^T @ ST (M=1 matmuls, col-packed to psum partitions 0/64);
  both PSUM accumulators are evacuated immediately (copy / reciprocal) so
  the next pair never stalls on the normalize chain; recips are partition-
  broadcast via a DRAM bounce (stride-0 partition DMA is DRAM-source only)
  and applied as one tensor_tensor multiply into AO.
  final out[q, do]: lhsT = AO tile, rhs = Wproj, f32 DMA out.

  The whole kernel is software-pipelined in 4 q-rounds of 512 columns:
  round qb interleaves attention (ACT-bound: 64 exps of [128,1024]) with
  the NEXT round's Q-projection matmuls and the PREVIOUS round's output-
  projection matmuls, so the PE work of those phases hides under the
  ScalarE exp stream. KT projection rides inside round 0; the V projection
  and first Q-projection block form the prologue.
"""

import sys

if "/opt/trn_rl_repo" not in sys.path:
    sys.path.insert(0, "/opt/trn_rl_repo")

import numpy as np
import ml_dtypes

from concourse import bacc, mybir, tile
from concourse.bass_utils import run_bass_kernel_spmd

# ---------------------------------------------------------------- constants
B, S, D = 2, 8192, 1024
H, HD, A = 16, 64, 1024
NQ = 2048          # token rows per core
NCORES = 8
DT = D // 128      # 8 x 128-row tiles of the model dim
AT = A // 128      # 8 anchor tiles
QB = 512           # q block inside attention
NPAIR = H // 2     # head pairs (adjacent heads share a 128-partition tile)
SCALE = 1.0 / 8.0  # 1/sqrt(hd)

F32 = mybir.dt.float32
BF16 = mybir.dt.bfloat16

_cached_nc = None

BIGEXP = False  # exp over [128, 2048] spanning two anchor tiles (slower)
COLSCORES = False  # scores as M=64-split col-mode mms (no PE mode switches)


def build_kernel(repeat=1):
    nc = bacc.Bacc("TRN2", target_bir_lowering=False, debug=False,
                   num_devices=NCORES)

    xT = nc.declare_dram_parameter("xT", [D, NQ], BF16, isOutput=False)
    aT = nc.declare_dram_parameter("aT", [D, A], BF16, isOutput=False)
    wq0 = nc.declare_dram_parameter("wq0", [D, D], BF16, isOutput=False)
    wq1 = nc.declare_dram_parameter("wq1", [D, D], BF16, isOutput=False)
    wkv = nc.declare_dram_parameter("wkv", [D, 2 * D], BF16, isOutput=False)
    wpr = nc.declare_dram_parameter("wpr", [D, D], BF16, isOutput=False)
    # Q bias, column 0 for q<1024 rows, column 1 for the rest (exact, f32)
    bq2 = nc.declare_dram_parameter("bq2", [D, 2], F32, isOutput=False)
    out = nc.declare_dram_parameter("out", [NQ, D], F32, isOutput=True)

    NQB = NQ // QB  # 4 q rounds

    with tile.TileContext(nc) as tc:
        for _rep in range(repeat):
          with (
            tc.tile_pool(name="attn", bufs=1) as p_attn,      # KT, V, QT
            tc.tile_pool(name="ao", bufs=1) as p_ao,          # AO blocks, WP
            tc.tile_pool(name="stage", bufs=1) as p_stage,    # aT
            tc.tile_pool(name="xq", bufs=2) as p_xq,          # x panel / round
            tc.tile_pool(name="wt", bufs=2) as p_w,           # weight panels
            tc.tile_pool(name="st", bufs=(2 if BIGEXP else 4)) as p_st,
            tc.tile_pool(name="small", bufs=1) as p_small,
            tc.tile_pool(name="pvs", bufs=4) as p_pvs,
            tc.tile_pool(name="rcb", bufs=3) as p_rcb,
            tc.tile_pool(name="scr", bufs=6, space="DRAM") as p_scr,
            tc.tile_pool(name="outsb", bufs=2) as p_out,
            tc.tile_pool(name="psps", bufs=(1 if BIGEXP else 2),
                         space="PSUM") as ps_s,
            tc.tile_pool(name="psacc", bufs=2, space="PSUM") as ps_acc,
            tc.tile_pool(name="psv", bufs=1, space="PSUM") as ps_v,
            tc.tile_pool(name="psd", bufs=1, space="PSUM") as ps_d,
          ):
            KT = p_attn.tile([128, DT, A], BF16, tag="KT")
            V = p_attn.tile([128, AT, D], BF16, tag="V")
            QT = p_attn.tile([128, DT, NQ], BF16, tag="QT")
            AOq = []
            for i in range(NQB):
                ao_i = p_ao.tile([128, DT, QB], BF16, tag=f"AO{i}",
                                 name=f"AO{i}")
                AOq.append(ao_i)
            WP = p_ao.tile([128, DT, D], BF16, tag="WP")
            ones = p_small.tile([128, 1], BF16, tag="ones")
            nc.vector.memset(ones[:], 1.0)
            bqs = p_small.tile([128, DT, 2], F32, tag="bqs")
            nc.sync.dma_start(
                out=bqs[:], in_=bq2[:].rearrange("(k p) c -> p k c", p=128))
            nc.gpsimd.dma_start(
                out=WP[:], in_=wpr[:].rearrange("(k p) c -> p k c", p=128))
            aTs = p_stage.tile([128, DT, A], BF16, tag="aT")
            nc.scalar.dma_start(
                out=aTs[:], in_=aT[:].rearrange("(k p) a -> p k a", p=128))

            # ---------------- emission helpers ---------------------------
            def emit_V_panel(vh):
                wv = p_w.tile([128, DT, 512], BF16, tag="wv", name=f"wv{vh}")
                nc.gpsimd.dma_start(
                    out=wv[:],
                    in_=wkv[:, D + vh * 512:D + (vh + 1) * 512].rearrange(
                        "(k p) c -> p k c", p=128))
                return wv

            def emit_V_at(vh, wv, at):
                acc = ps_acc.tile([128, 512], F32, tag="acc")
                for dn in range(DT):
                    nc.tensor.matmul(
                        acc[:],
                        lhsT=aTs[:, dn, at * 128:(at + 1) * 128],
                        rhs=wv[:, dn, :],
                        start=(dn == 0), stop=(dn == DT - 1))
                nc.vector.tensor_copy(
                    V[:, at, vh * 512:(vh + 1) * 512], acc[:])

            def emit_KT_dma(dk):
                wk = p_w.tile([128, DT, 128], BF16, tag="wk")
                nc.gpsimd.dma_start(
                    out=wk[:],
                    in_=wkv[:, dk * 128:(dk + 1) * 128].rearrange(
                        "(k p) c -> p k c", p=128))
                return wk

            def KT_chunks(dk, wk):
                """KT projection for block dk as 8 filler chunks of 2 mms."""
                state = {}
                chunks = []
                for ah in range(2):
                    for i in range(4):
                        def f(ah=ah, i=i):
                            if i == 0:
                                state[ah] = ps_acc.tile(
                                    [128, 512], F32, tag="acc",
                                    name=f"acc_kt{dk}_{ah}")
                            acc = state[ah]
                            for dn in (2 * i, 2 * i + 1):
                                nc.tensor.matmul(
                                    acc[:],
                                    lhsT=wk[:, dn, :],
                                    rhs=aTs[:, dn, ah * 512:(ah + 1) * 512],
                                    start=(dn == 0), stop=(dn == DT - 1))
                            if i == 3:
                                nc.vector.tensor_copy(
                                    KT[:, dk, ah * 512:(ah + 1) * 512],
                                    acc[:])
                        chunks.append(f)
                return chunks

            def emit_xq(qs):
                xq = p_xq.tile([128, DT, QB], BF16, tag="xq")
                nc.scalar.dma_start(
                    out=xq[:],
                    in_=xT[:, qs * QB:(qs + 1) * QB].rearrange(
                        "(k p) q -> p k q", p=128))
                return xq

            def emit_C_dma(qs, dq):
                half = 0 if qs < 2 else 1
                wsrc = wq0 if half == 0 else wq1
                wq_t = p_w.tile([128, DT, 128], BF16, tag="wq")
                nc.gpsimd.dma_start(
                    out=wq_t[:],
                    in_=wsrc[:, dq * 128:(dq + 1) * 128].rearrange(
                        "(k p) c -> p k c", p=128))
                return wq_t

            def C_chunks(qs, dq, xq, wq_t):
                """Q-projection block (qs, dq) as 4 filler chunks of 2 mms."""
                half = 0 if qs < 2 else 1
                state = {}
                chunks = []
                for i in range(4):
                    def f(i=i):
                        if i == 0:
                            state["acc"] = ps_acc.tile(
                                [128, 512], F32, tag="acc",
                                name=f"acc_c{qs}_{dq}")
                        acc = state["acc"]
                        for dn in (2 * i, 2 * i + 1):
                            nc.tensor.matmul(
                                acc[:],
                                lhsT=wq_t[:, dn, :],
                                rhs=xq[:, dn, :],
                                start=(dn == 0), stop=(dn == DT - 1))
                        if i == 3:
                            nc.vector.tensor_scalar_add(
                                QT[:, dq, qs * QB:(qs + 1) * QB], acc[:],
                                bqs[:, dq, half:half + 1])
                    chunks.append(f)
                return chunks

            def emit_scores_exp(qb, g, at):
                ps = ps_s.tile([128, 2 * QB], F32, tag="ps")
                for hl in range(2):
                    nc.tensor.matmul(
                        ps[:, hl * QB:(hl + 1) * QB],
                        lhsT=KT[hl * 64:(hl + 1) * 64, g,
                                at * 128:(at + 1) * 128],
                        rhs=QT[hl * 64:(hl + 1) * 64, g,
                               qb * QB:(qb + 1) * QB],
                        start=True, stop=True)
                st = p_st.tile([128, 2 * QB], BF16, tag="st")
                nc.scalar.activation(
                    st[:], ps[:], mybir.ActivationFunctionType.Exp,
                    scale=SCALE)
                return st

            def emit_av_denom(g, at, st, pv, pd):
                for hl in range(2):
                    h = 2 * g + hl
                    nc.tensor.matmul(
                        pv[hl * 64:(hl + 1) * 64, :],
                        lhsT=V[:, at, h * 64:(h + 1) * 64],
                        rhs=st[:, hl * QB:(hl + 1) * QB],
                        start=(at == 0), stop=(at == AT - 1),
                        tile_position=(0, hl * 64))
                for hl in range(2):
                    nc.tensor.matmul(
                        pd[hl * 64:hl * 64 + 1, :],
                        lhsT=ones[:],
                        rhs=st[:, hl * QB:(hl + 1) * QB],
                        start=(at == 0), stop=(at == AT - 1),
                        tile_position=(0, hl * 64))

            def emit_normalize(qb, g, pv, pd):
                pvs = p_pvs.tile([128, QB], F32, tag="pvs")
                nc.vector.tensor_copy(pvs[:], pv[:])
                rc = p_pvs.tile([65, QB], F32, tag="rcp")
                nc.vector.reciprocal(rc[:], pd[0:65, :])
                scr = p_scr.tile([2, QB], F32, tag="scr")
                nc.sync.dma_start(out=scr[0:1, :], in_=rc[0:1, :])
                nc.sync.dma_start(out=scr[1:2, :], in_=rc[64:65, :])
                rb = p_rcb.tile([128, QB], F32, tag="rb")
                for hl in range(2):
                    nc.sync.dma_start(
                        out=rb[hl * 64:(hl + 1) * 64, :],
                        in_=scr[hl:hl + 1, :].to_broadcast((64, QB)))
                nc.vector.tensor_tensor(
                    out=AOq[qb][:, g, :], in0=pvs[:], in1=rb[:],
                    op=mybir.AluOpType.mult)

            def emit_attention(qb, g, fillers=(), at_hook=None):
                """One head-pair attention step, fillers interleaved.

                Per at-slot: scores pair (row-tiled, concurrent), then the
                LAGGED av+denom for at-1 (so it never waits on ScalarE's
                exp), then a slice of the filler closures (projection /
                KT / V matmul chunks) to soak the PE while ScalarE chews
                through the exp stream.
                """
                pv = ps_v.tile([128, QB], F32, tag="pv")
                pd = ps_d.tile([128, QB], F32, tag="pd")
                fillers = list(fillers)
                nf = len(fillers)
                fi = 0
                sts = []
                for at in range(AT):
                    sts.append(emit_scores_exp(qb, g, at))
                    if at_hook is not None:
                        at_hook(at)
                    if at > 0:
                        emit_av_denom(g, at - 1, sts[at - 1], pv, pd)
                    tgt = (at + 1) * nf // AT
                    while fi < tgt:
                        fillers[fi]()
                        fi += 1
                emit_av_denom(g, AT - 1, sts[AT - 1], pv, pd)
                emit_normalize(qb, g, pv, pd)

            def E_chunks(qbi, qi):
                """Out-projection of tile (qbi, qi) as 8 chunks of 2 mms."""
                state = {}
                chunks = []
                for dh in range(2):
                    for i in range(4):
                        def f(dh=dh, i=i):
                            if dh == 0 and i == 0:
                                state["ot"] = p_out.tile(
                                    [128, D], F32, tag="ot",
                                    name=f"ot{qbi}_{qi}")
                            if i == 0:
                                state["acc"] = ps_acc.tile(
                                    [128, 512], F32, tag="acc",
                                    name=f"acc_e{qbi}_{qi}_{dh}")
                            acc = state["acc"]
                            for dv in (2 * i, 2 * i + 1):
                                nc.tensor.matmul(
                                    acc[:],
                                    lhsT=AOq[qbi][:, dv,
                                                  qi * 128:(qi + 1) * 128],
                                    rhs=WP[:, dv, dh * 512:(dh + 1) * 512],
                                    start=(dv == 0), stop=(dv == DT - 1))
                            if i == 3:
                                nc.vector.tensor_copy(
                                    state["ot"][:, dh * 512:(dh + 1) * 512],
                                    acc[:])
                                if dh == 1:
                                    qt = qbi * (QB // 128) + qi
                                    nc.sync.dma_start(
                                        out=out[qt * 128:(qt + 1) * 128, :],
                                        in_=state["ot"][:])
                        chunks.append(f)
                return chunks

            # ---------------- software-pipelined schedule ----------------
            # Every g-step interleaves its attention at-loop with filler
            # chunks: the JIT Q-projection C(qb, g+1), round-0's KT(g+1)
            # and vh=1 V blocks, and rounds>=1's out-projection of the
            # previous round.  Weight DMAs for a step's fillers are issued
            # one full g-step ahead so the chunks never wait on HBM.
            def c_target(qb, g):
                """C block computed during step (qb, g): two steps ahead,
                so the QT write never races the consuming scores."""
                s = qb * NPAIR + g + 2
                if s < NQB * NPAIR:
                    return (s // NPAIR, s % NPAIR)
                return None

            wvs = [emit_V_panel(0), emit_V_panel(1)]
            xqs = [emit_xq(0)]
            wkts = {0: emit_KT_dma(0)}
            wqts = {(0, 0): emit_C_dma(0, 0), (0, 1): emit_C_dma(0, 1)}
            for f in KT_chunks(0, wkts[0]):
                f()
            for f in C_chunks(0, 0, xqs[0], wqts[(0, 0)]):
                f()
            for f in C_chunks(0, 1, xqs[0], wqts[(0, 1)]):
                f()
            wkts[1] = emit_KT_dma(1)
            wqts[(0, 2)] = emit_C_dma(0, 2)
            # vh=1 V blocks are first needed at g=4; spread them over g=1..3
            v1_assign = {1: (0, 1, 2), 2: (3, 4, 5), 3: (6, 7)}
            e_rr = []
            for qb in range(NQB):
                if qb + 1 < NQB:
                    xqs.append(emit_xq(qb + 1))
                if qb >= 1:
                    e_rr = [c for qi in range(QB // 128)
                            for c in E_chunks(qb - 1, qi)]
                for g in range(NPAIR):
                    # prefetch weight panels for the NEXT step's fillers
                    ns = (qb, g + 1) if g + 1 < NPAIR else (
                        (qb + 1, 0) if qb + 1 < NQB else None)
                    if ns is not None:
                        t2 = c_target(*ns)
                        if t2 is not None and t2 not in wqts:
                            wqts[t2] = emit_C_dma(*t2)
                        if ns[0] == 0 and ns[1] + 1 < NPAIR \
                                and (ns[1] + 1) not in wkts:
                            wkts[ns[1] + 1] = emit_KT_dma(ns[1] + 1)
                    # fillers for THIS step (KT first: needed by step g+1;
                    # the C block is two steps out so it can ride later)
                    fillers = []
                    if qb == 0 and g + 1 < NPAIR:
                        fillers += KT_chunks(g + 1, wkts[g + 1])
                    t = c_target(qb, g)
                    if t is not None:
                        fillers += C_chunks(t[0], t[1], xqs[t[0]], wqts[t])
                    if qb == 0 and g in v1_assign:
                        for at_ in v1_assign[g]:
                            fillers.append(
                                lambda at_=at_: emit_V_at(1, wvs[1], at_))
                    if qb >= 1:
                        fillers += e_rr[g * 4:(g + 1) * 4]
                    at_hook = None
                    if qb == 0 and g == 0:
                        at_hook = lambda at: emit_V_at(0, wvs[0], at)
                    emit_attention(qb, g, fillers, at_hook)
            for qi in range(QB // 128):
                for f in E_chunks(NQB - 1, qi):
                    f()

    nc.compile()
    return nc


def shard_inputs(x, Wqkv, Wq, Wproj, bqkv, bq):
    """Build per-core in_maps (bf16, pre-transposed; biases f32)."""
    bf = ml_dtypes.bfloat16
    xtb = np.ascontiguousarray(np.transpose(x, (0, 2, 1))).astype(bf)  # (B,D,S)
    wq_anchor = np.ascontiguousarray(Wqkv[:, :D]).astype(bf)
    wq_plain = np.ascontiguousarray(Wq).astype(bf)
    wkv_b = np.ascontiguousarray(Wqkv[:, D:3 * D]).astype(bf)
    wpr_b = np.ascontiguousarray(Wproj).astype(bf)
    bq_anchor = np.asarray(bqkv[:D], np.float32)
    bq_plain = np.asarray(bq, np.float32)
    bq2_anchor = np.ascontiguousarray(
        np.stack([bq_anchor, bq_plain], axis=1))
    bq2_plain = np.ascontiguousarray(
        np.stack([bq_plain, bq_plain], axis=1))
    in_maps = []
    for c in range(NCORES):
        b, blk = c // 4, c % 4
        q0 = blk * NQ
        in_maps.append({
            "xT": np.ascontiguousarray(xtb[b, :, q0:q0 + NQ]),
            "aT": np.ascontiguousarray(xtb[b, :, :A]),
            "wq0": wq_anchor if blk == 0 else wq_plain,
            "wq1": wq_plain,
            "wkv": wkv_b,
            "wpr": wpr_b,
            "bq2": bq2_anchor if blk == 0 else bq2_plain,
        })
    return in_maps


def kernel(x, Wqkv, bqkv, Wq, bq, Wproj, bproj, num_anchor_tokens):
    global _cached_nc
    x = np.asarray(x, dtype=np.float32)
    Wqkv = np.asarray(Wqkv, dtype=np.float32)
    Wq = np.asarray(Wq, dtype=np.float32)
    Wproj = np.asarray(Wproj, dtype=np.float32)
    assert int(num_anchor_tokens) == A and x.shape == (B, S, D)

    bqkv = np.asarray(bqkv, dtype=np.float32)
    bq = np.asarray(bq, dtype=np.float32)
    bproj = np.asarray(bproj, dtype=np.float32)
    in_maps = shard_inputs(x, Wqkv, Wq, Wproj, bqkv, bq)
    if _cached_nc is None:
        _cached_nc = build_kernel()

    def run_once():
        res = run_bass_kernel_spmd(_cached_nc, in_maps,
                                   core_ids=list(range(NCORES)))
        o = np.empty((B, S, D), dtype=np.float32)
        for c in range(NCORES):
            b, blk = c // 4, c % 4
            o[b, blk * NQ:(blk + 1) * NQ, :] = res.results[c]["out"]
        # K bias cancels in softmax (constant per-query score shift).
        # V bias adds bv to every attention output -> exact bv@Wproj add.
        o += bqkv[2 * D:3 * D] @ Wproj
        o += bproj
        return o

    # one-row host probe guards against rare transient device corruption
    def probe_err(o):
        r = A  # first non-anchor row of batch 0
        anch = x[0, :A, :]
        Km = anch @ Wqkv[:, D:2 * D] + bqkv[D:2 * D]
        Vm = anch @ Wqkv[:, 2 * D:3 * D] + bqkv[2 * D:3 * D]
        q = x[0, r, :] @ Wq + bq
        row = np.empty(D, np.float32)
        for h in range(H):
            sl = slice(h * HD, (h + 1) * HD)
            s = (Km[:, sl] @ q[sl]) * SCALE
            e = np.exp(s - s.max())
            row[sl] = (e / e.sum()) @ Vm[:, sl]
        ref_row = row @ Wproj + bproj
        return (np.linalg.norm(o[0, r] - ref_row)
                / max(np.linalg.norm(ref_row), 1e-6))

    out = run_once()
    if not np.isfinite(out).all() or probe_err(out) > 5e-2:
        out = run_once()
    return out

